# revision 5
# baseline (speedup 1.0000x reference)
"""TRN2 Bass kernel v4 for nn_CVAEWithTrajectoryOptimization.

Same math as the v3 baseline (Sherman-Morrison LM: delta =
-e*g/(damping+||g||^2), 8 serial fwd+bwd MLP iterations), rebuilt around
HW-measured instruction costs (For_i trip-count-slope microbenches):
  - dependent (chained) DVE/ACT ops cost ~230-430 ns EACH regardless of
    engine; cross-engine hops add nothing beyond that -> the only levers
    are CHAIN OP COUNT and full-128-partition shapes (sub-128 partition
    DVE ops pay ~200 ns extra; everything action-space is padded 112->128).
  - MM instruction cost: f16/bf16 with 128-col stationary (FWL)
    ~44/34 ns, 112-col stationary ~141 ns, fp32 ~444 ns -> no fp32 MMs
    anywhere, all stationaries padded to 128 cols.
Design:
  - biases enter through the PE: identity-stationary matmuls stream the
    f16/bf16 hi+lo split of c1bT / b2bT into the same PSUM accumulation
    group as the weight matmuls, so t1/t2 are assembled entirely on PE
    and ACT reads PSUM directly (no DVE bias-add chain ops).
  - elu' gate em = exp(min(t,0)) = Exp(-Relu(-t)): two in-order ACT ops
    reading PSUM; h = elu(t)+1 = max(t+1, em) (one DVE op; t+1 prepared
    off-chain; the +1 is absorbed into b2bT = b2 - colsum(W2)).
  - ||g||^2 + damping in ONE chained ACT op: Square with accum_out over
    gT[128,33] whose 33rd column holds sqrt(DAMP/128); e-path offset E0P
    rides the escr Identity-activation bias; ones-matmul broadcasts both
    column sums to all partitions (partition-reduce + bcast in one MM).
  - precision schedule 'phhhhhhh': iteration 0 (the bifurcation-critical
    one: |upd| up to 3.6 parks ~1/3 of actions just past the +-1 clip
    boundary) runs bf16 STATIONARY hi/lo x STREAM hi/lo triplet matmuls
    (~fp32 grade, rel 4.8e-3 vs 6.0e-3 for v3's fp32-x2 schedule) at
    ~1/10 the fp32 MM cost; iterations 1-7 are f16 (f16-solo = 1.9e-2,
    bf16-solo streams = 2.4e-2 -> measured, not guessed).
  - Exp/Relu/Abs/Sign/Identity/Square all live in the one
    'exp_and_others' ACT table set -> no table reloads.
  - clip-grad mask emitted after em1 (it head-of-line blocked a1 on the
    in-order ACT queue); iteration-0's stream hi/lo split is computed
    HOST-side (init_actions is an input) and DMA'd; the solve accums are
    cast once to f16 for a 44 ns f16 ones-MM (vs 444 ns fp32).
  - PSUM constraint (hard-won): accumulation groups must stay contiguous
    per bank region -- pre-opening several regions' bias groups to hoist
    them into stall windows silently corrupts results.
Measured (For_i trip-count slope, marginal ns per full kernel exec,
8-core SPMD): v3 baseline 132.9 us (early session) / 88-118 us (later
machine states); this kernel 70.8-83 us; paired same-session ratios
0.56-0.82, median ~0.76.
Replicated on all 8 cores (serial latency-bound chain; collectives would
dominate any sharding win).
"""
import os
import numpy as np

_B, _HH, _AA = 32, 16, 7
_HA = _HH * _AA          # 112
_SZ = 576
_NF = 512
_DAMP, _STEP, _ITERS, _OFF = 0.1, 0.1, 8, 1000.0
_N_CORES = 8
_PRIO_LOW = 1_500_000_000

# precision schedule: 'p' = bf16 hi/lo stationary pairs + hi/lo split
# streams (~fp32), 'q' = bf16 pairs + single-bf16 streams, 'h' = f16
_PRECS = os.environ.get("V4_PRECS", "phhhhhhh")

# f16 blob columns
_C16_W1A = 0
_C16_W2 = 512
_C16_W2TW = 2560
_C16_W1ATP = 4608
_C16_W3C = 5120
_C16_EYE = 5124
_C16_C1H = 5252
_C16_C1L = 5380
_C16_B2H = 5508
_C16_B2L = 5636
_C16 = 5764
# bf16 pair blob columns (hi interleaved before lo per tensor)
_CB_W1AH = 0
_CB_W1AL = 512
_CB_W2H = 1024
_CB_W2L = 3072
_CB_W2TWH = 5120
_CB_W2TWL = 7168
_CB_W1ATPH = 9216
_CB_W1ATPL = 9728
_CB_W3CH = 10240
_CB_EYE = 10244
_CB_C1H = 10372
_CB_C1L = 10500
_CB_B2H = 10628
_CB_B2L = 10756
_CB = 10884
# f32 blob columns (tiny)
_CF = 0

_CACHE = {}


def _dchunks(total, n):
    """Split [0,total) into n contiguous col ranges."""
    step = (total + n - 1) // n
    return [(i, min(i + step, total)) for i in range(0, total, step)]


def _emit_state(nc, tc, sb, ps, D, mybir, precs):
    f32 = mybir.dt.float32
    f16 = mybir.dt.float16
    bf16 = mybir.dt.bfloat16
    S = {}
    # flatT padded to 128 partitions (rows 112-127 stay zero): sub-128
    # partition-dim DVE ops cost ~434 ns vs ~231 ns at 128.
    S["flatT"] = sb.tile([128, _B], f32, tag="flatT", name="flatT")
    nc.vector.memset(S["flatT"][:], 0.0)
    nc.sync.dma_start(S["flatT"][0:_HA, :], D["flatT0"])
    if precs[0] == "p":
        # iteration 0's stream hi/lo split comes straight from the host
        # (init_actions is an input; |init| < 1 so clip is identity)
        S["act0h"] = sb.tile([128, _B], bf16, tag="act0h", name="act0h")
        S["act0l"] = sb.tile([128, _B], bf16, tag="act0l", name="act0l")
        nc.scalar.dma_start(S["act0h"][:], D["ACT0H"])
        nc.scalar.dma_start(S["act0l"][:], D["ACT0L"])

    queues = [nc.sync, nc.scalar, nc.gpsimd]
    qi = 0

    def q_dma(dst, src):
        nonlocal qi
        queues[qi % len(queues)].dma_start(dst, src)
        qi += 1

    has_q = ("q" in precs) or ("p" in precs)
    if has_q:
        blobb = sb.tile([128, _CB], bf16, tag="blobb", name="blobb")
        for a, b in _dchunks(_CB, 10):
            q_dma(blobb[:, a:b], D["BLOBB"][:, a:b])
        S["w1a_qh"] = blobb[:, _CB_W1AH:_CB_W1AH + _NF]
        S["w1a_ql"] = blobb[:, _CB_W1AL:_CB_W1AL + _NF]
        S["w2_qh"] = [blobb[:, _CB_W2H + _NF*k:_CB_W2H + _NF*(k+1)]
                      for k in range(4)]
        S["w2_ql"] = [blobb[:, _CB_W2L + _NF*k:_CB_W2L + _NF*(k+1)]
                      for k in range(4)]
        S["w2tw_qh"] = [blobb[:, _CB_W2TWH + _NF*k:_CB_W2TWH + _NF*(k+1)]
                        for k in range(4)]
        S["w2tw_ql"] = [blobb[:, _CB_W2TWL + _NF*k:_CB_W2TWL + _NF*(k+1)]
                        for k in range(4)]
        S["w1atp_qh"] = [blobb[:, _CB_W1ATPH + 128*k:_CB_W1ATPH + 128*(k+1)]
                         for k in range(4)]
        S["w1atp_ql"] = [blobb[:, _CB_W1ATPL + 128*k:_CB_W1ATPL + 128*(k+1)]
                         for k in range(4)]
        S["w3c_q"] = blobb[:, _CB_W3CH:_CB_W3CH + 4]
        S["eye_q"] = blobb[:, _CB_EYE:_CB_EYE + 128]
        S["c1h_q"] = blobb[:, _CB_C1H:_CB_C1H + 128]
        S["c1l_q"] = blobb[:, _CB_C1L:_CB_C1L + 128]
        S["b2h_q"] = blobb[:, _CB_B2H:_CB_B2H + 128]
        S["b2l_q"] = blobb[:, _CB_B2L:_CB_B2L + 128]

    if "h" in precs:
        blob16 = sb.tile([128, _C16], f16, tag="blob16", name="blob16")
        for a, b in _dchunks(_C16, 6):
            q_dma(blob16[:, a:b], D["BLOB16"][:, a:b])
        S["w1a_h"] = blob16[:, _C16_W1A:_C16_W1A + _NF]
        S["w2_h"] = [blob16[:, _C16_W2 + _NF*k:_C16_W2 + _NF*(k+1)]
                     for k in range(4)]
        S["w2tw_h"] = [blob16[:, _C16_W2TW + _NF*k:_C16_W2TW + _NF*(k+1)]
                       for k in range(4)]
        S["w1atp_h"] = [blob16[:, _C16_W1ATP + 128*k:_C16_W1ATP + 128*(k+1)]
                        for k in range(4)]
        S["w3c_h"] = blob16[:, _C16_W3C:_C16_W3C + 4]
        S["eye_h"] = blob16[:, _C16_EYE:_C16_EYE + 128]
        S["c1h_h"] = blob16[:, _C16_C1H:_C16_C1H + 128]
        S["c1l_h"] = blob16[:, _C16_C1L:_C16_C1L + 128]
        S["b2h_h"] = blob16[:, _C16_B2H:_C16_B2H + 128]
        S["b2l_h"] = blob16[:, _C16_B2L:_C16_B2L + 128]

    S["ones"] = sb.tile([128, 128], f16, tag="ones", name="ones")
    nc.vector.memset(S["ones"][:], 1.0)
    S["rhs16"] = sb.tile([128, 2], f16, tag="rhs16", name="rhs16")
    # rhs_ge [128, 2] f32: col0 = per-partition sum(g^2) partials (ACT
    # Square accum over gT[128,33]; gT col 32 holds sqrt(DAMP/128) so the
    # damping rides the same accumulation); col1 row0 = sum(p_r)*STEP/B
    # + E0P (E0P rides the escr bias).  ones-MM broadcasts col sums.
    S["rhs_ge"] = sb.tile([128, 2], f32, tag="rhs_ge", name="rhs_ge")
    nc.vector.memset(S["rhs_ge"][:], 0.0)
    S["deb"] = sb.tile([1, 1], f32, tag="deb", name="deb")
    nc.sync.dma_start(S["deb"][:], D["DE"])
    S["gT"] = sb.tile([128, _B + 1], f32, tag="gT", name="gT")
    nc.vector.memset(S["gT"][:, _B:_B+1],
                     float(np.sqrt(_DAMP / 128.0)))

    S["p_h1"] = ps.tile([128, 128], f32, tag="p_h1", name="p_h1")
    S["p_h2"] = ps.tile([128, 128], f32, tag="p_h2", name="p_h2")
    S["p_g1"] = ps.tile([128, 128], f32, tag="p_g1", name="p_g1")
    S["p_ga"] = ps.tile([128, _B], f32, tag="p_ga", name="p_ga")
    S["p_r"] = ps.tile([1, _B], f32, tag="p_r", name="p_r")
    S["p_ge"] = ps.tile([128, 2], f32, tag="p_ge", name="p_ge")
    S["p_scr"] = ps.tile([_B, 1], f32, tag="p_scr", name="p_scr")
    S["nprio"] = 0

    # PE clock warm across the DMA window; ACT Exp table pre-warm
    warm_deps = [S["flatT"][0:128, 0:_B]]
    if has_q:
        warm_deps += [S["w1a_qh"][0:112, 0:32], S["w2_qh"][3][0:112, 0:32]]
    if "h" in precs:
        warm_deps += [S["w2_h"][3][0:112, 0:32]]
    for dep in warm_deps:
        for _ in range(8):
            _dummy_mm(nc, S, dep)
    warm = sb.tile([1, 1], f32, tag="actwarm", name="actwarm")
    a1 = nc.scalar.activation(warm[:], S["rhs_ge"][0:1, 0:1],
                              mybir.ActivationFunctionType.Exp)
    a1.bass_priority = _PRIO_LOW - 2
    return S


def _emit_bias1(nc, S, wp):
    """fwd1 bias: 2 identity-stationary MMs per m-region starting the
    p_h1 PSUM groups.  Emitted in the PREVIOUS iteration's tail window
    (PE is idle there) so fwd1 only runs 4 weight MMs after the clip."""
    eye = S[f"eye_{wp}"]
    for m in range(4):
        reg = S["p_h1"][:, 32*m:32*m+32]
        nc.tensor.matmul(reg, eye, S[f"c1h_{wp}"][:, 32*m:32*m+32],
                         start=True, stop=False)
        nc.tensor.matmul(reg, eye, S[f"c1l_{wp}"][:, 32*m:32*m+32],
                         start=False, stop=False)


def _dummy_mm(nc, S, dep):
    m = dep.shape[1] if len(dep.shape) > 1 else 1
    mm = nc.tensor.matmul(S["p_scr"][0:m, :], dep, dep[:, 0:1],
                          start=True, stop=True)
    mm.bass_priority = _PRIO_LOW + S["nprio"]
    S["nprio"] += 1
    return mm


def _emit_iter(nc, S, sb, mybir, prec, nxt_prec, first=False,
               emit_next_bias1=True):
    """One LM iteration.  prec: 'p' = bf16 pairs + split streams,
    'q' = bf16 pairs + single-bf16 streams, 'h' = f16.
    first=True: |init_actions| < 1 so the clip-grad mask is all-ones ->
    no mask, gT = p_ga directly."""
    f32 = mybir.dt.float32
    f16 = mybir.dt.float16
    bf16 = mybir.dt.bfloat16
    pair = prec in ("q", "p")
    split = prec == "p"
    wp = "q" if pair else "h"          # weight-blob key
    dt = bf16 if pair else f16
    Alu = mybir.AluOpType
    Act = mybir.ActivationFunctionType
    flatT = S["flatT"]
    p_h1, p_h2, p_g1 = S["p_h1"], S["p_h2"], S["p_g1"]
    p_ga, p_r, p_ge = S["p_ga"], S["p_r"], S["p_ge"]

    def t(name, shape, d):
        return sb.tile(shape, d, tag=f"{name}_{prec}", name=f"{name}_{prec}")

    def mm_pairs(psum_ap, stat_hi, stat_lo, stream, start, stop,
                 stream_lo=None):
        if not pair:
            nc.tensor.matmul(psum_ap, stat_hi, stream, start=start, stop=stop)
            return
        nc.tensor.matmul(psum_ap, stat_hi, stream, start=start, stop=False)
        if stream_lo is not None:
            nc.tensor.matmul(psum_ap, stat_hi, stream_lo, start=False,
                             stop=False)
        nc.tensor.matmul(psum_ap, stat_lo, stream, start=False, stop=stop)

    def w(name, k=None):
        if pair:
            if k is None:
                return S[f"{name}_qh"], S[f"{name}_ql"]
            return S[f"{name}_qh"][k], S[f"{name}_ql"][k]
        if k is None:
            return S[f"{name}_h"], None
        return S[f"{name}_h"][k], None

    def split_hl(x32, nm):
        """f32 -> (bf16 hi, bf16 lo) pair; lo=None when not splitting."""
        if not split:
            return None, None
        xh = t(nm + "H", list(x32.shape), bf16)
        xl = t(nm + "L", list(x32.shape), bf16)
        nc.vector.tensor_scalar_mul(xh[:], x32, 1.0)
        nc.vector.tensor_tensor(xl[:], x32, xh[:], op=Alu.subtract)
        return xh, xl

    # head: acts = clip(flat) -> stream dtype (hi/lo split for 'p')
    if split and first:
        actsT = S["act0h"]          # host-computed bf16 hi/lo of init
        actsL = S["act0l"]
    elif split:
        acts32 = t("acts32", [128, _B], f32)
        nc.vector.tensor_scalar(acts32[:], flatT[:], -1.0, 1.0,
                                op0=Alu.max, op1=Alu.min)
        actsT = t("actsT", [128, _B], dt)
        actsL = t("actsL", [128, _B], dt)
        nc.vector.tensor_scalar_mul(actsT[:], acts32[:], 1.0)
        nc.vector.tensor_tensor(actsL[:], acts32[:], actsT[:],
                                op=Alu.subtract)
    else:
        actsT = t("actsT", [128, _B], dt)
        actsL = None
        nc.vector.tensor_scalar(actsT[:], flatT[:], -1.0, 1.0,
                                op0=Alu.max, op1=Alu.min)

    # fwd1: per m-region, bias pair then weight MMs (PSUM allows only one
    # open accumulation group per bank region sequence -- keep contiguous)
    eye = S[f"eye_{wp}"]
    w1h, w1l = w("w1a")
    for m in range(4):
        reg = p_h1[:, 32*m:32*m+32]
        nc.tensor.matmul(reg, eye, S[f"c1h_{wp}"][:, 32*m:32*m+32],
                         start=True, stop=False)
        nc.tensor.matmul(reg, eye, S[f"c1l_{wp}"][:, 32*m:32*m+32],
                         start=False, stop=False)
        mm_pairs(reg,
                 w1h[:, 128*m:128*(m+1)],
                 w1l[:, 128*m:128*(m+1)] if w1l is not None else None,
                 actsT[:], start=False, stop=True,
                 stream_lo=actsL[:] if actsL is not None else None)

    # em1 = exp(min(t1,0)) = Exp(-Relu(-t1)); h1s = max(t1+1, em1)
    hdt = f32 if split else dt
    a1 = t("a1", [128, 128], f32)
    em1 = t("em1", [128, 128], hdt)
    t1p1 = t("t1p1", [128, 128], hdt)
    h1s = t("h1s", [128, 128], hdt)
    nc.scalar.activation(a1[:], p_h1[:], Act.Relu, scale=-1.0)
    nc.scalar.activation(em1[:], a1[:], Act.Exp, scale=-1.0)
    nc.vector.tensor_scalar_add(t1p1[:], p_h1[:], 1.0)
    nc.vector.tensor_tensor(h1s[:], t1p1[:], em1[:], op=Alu.max)
    h1sH, h1sL = split_hl(h1s[:], "h1s")
    if split:
        h1s = h1sH

    # clip-grad mask from the CURRENT flatT (emitted after em1 so it no
    # longer head-of-line blocks a1 on the in-order ACT queue; correctness
    # is positional: it reads flatT before this iteration's update)
    if not first:
        absT = t("absT", [128, _B], f32)
        sgnT = t("sgnT", [128, _B], f32)
        maskT = t("maskT", [128, _B], f16)
        nc.scalar.activation(absT[:], flatT[:], Act.Abs)
        nc.scalar.activation(sgnT[:], absT[:], Act.Sign, bias=1.0, scale=-1.0)
        nc.scalar.activation(maskT[:], sgnT[:], Act.Relu)

    # fwd2: per m-region, bias pair then weight MMs
    for m in range(4):
        reg = p_h2[:, 32*m:32*m+32]
        nc.tensor.matmul(reg, eye, S[f"b2h_{wp}"][:, 32*m:32*m+32],
                         start=True, stop=False)
        nc.tensor.matmul(reg, eye, S[f"b2l_{wp}"][:, 32*m:32*m+32],
                         start=False, stop=False)
        for k in range(4):
            h, lo = w("w2", k)
            mm_pairs(reg,
                     h[:, 128*m:128*(m+1)],
                     lo[:, 128*m:128*(m+1)] if lo is not None else None,
                     h1s[:, 32*k:32*k+32], start=False, stop=(k == 3),
                     stream_lo=h1sL[:, 32*k:32*k+32] if h1sL is not None
                     else None)

    # em2 = exp(min(t2,0))
    a2 = t("a2", [128, 128], f32)
    em2 = t("em2", [128, 128], hdt)
    nc.scalar.activation(a2[:], p_h2[:], Act.Relu, scale=-1.0)
    nc.scalar.activation(em2[:], a2[:], Act.Exp, scale=-1.0)
    em2H, em2L = split_hl(em2[:], "em2")
    em2s = em2H if split else em2

    # bwd2: p_g1 = W2TW^T-chunks @ em2   (W3/B scale folded host-side)
    for m in range(4):
        for k in range(4):
            h, lo = w("w2tw", k)
            mm_pairs(p_g1[:, 32*m:32*m+32],
                     h[:, 128*m:128*(m+1)],
                     lo[:, 128*m:128*(m+1)] if lo is not None else None,
                     em2s[:, 32*k:32*k+32], start=(k == 0), stop=(k == 3),
                     stream_lo=em2L[:, 32*k:32*k+32] if em2L is not None
                     else None)

    # reward path (off-chain): t2p1/h2s fill the DVE while PE runs bwd2
    t2p1 = t("t2p1", [128, 128], dt)
    h2s = t("h2s", [128, 128], dt)
    nc.vector.tensor_scalar_add(t2p1[:], p_h2[:], 1.0)
    nc.vector.tensor_tensor(h2s[:], t2p1[:], em2[:], op=Alu.max)
    w3 = S[f"w3c_{wp}"]
    for k in range(4):
        nc.tensor.matmul(p_r[:], w3[:, k:k+1], h2s[:, 32*k:32*k+32],
                         start=(k == 0), stop=(k == 3))
    # e-path on ACT: rhs_ge[0,1] = sum(p_r)*STEP/B + E0P (via bias)
    escr = t("escr", [1, _B], f32)
    nc.scalar.activation(escr[:], p_r[:], Act.Identity,
                         bias=S["deb"][:],
                         scale=float(np.float32(_STEP / _B)),
                         accum_out=S["rhs_ge"][0:1, 1:2])

    # bwd chain: gh1p = p_g1 * em1; bwd1; gT = p_ga * mask
    gh1p = t("gh1p", [128, 128], hdt)
    nc.vector.tensor_tensor(gh1p[:], p_g1[:], em1[:], op=Alu.mult)
    gh1pH, gh1pL = split_hl(gh1p[:], "gh1p")
    gh1ps = gh1pH if split else gh1p
    for k in range(4):
        h, lo = w("w1atp", k)
        mm_pairs(p_ga[:], h, lo, gh1ps[:, 32*k:32*k+32],
                 start=(k == 0), stop=(k == 3),
                 stream_lo=gh1pL[:, 32*k:32*k+32] if gh1pL is not None
                 else None)

    gTt = S["gT"]
    if first:
        nc.vector.tensor_scalar_mul(gTt[:, 0:_B], p_ga[:], 1.0)
    else:
        nc.vector.tensor_tensor(gTt[:, 0:_B], p_ga[:], maskT[:],
                                op=Alu.mult)
    gT = gTt[:, 0:_B]

    # ||g||^2 + DAMP via ACT Square accum over [128, 33] -> rhs_ge col0
    sqd = t("sqd", [128, _B + 1], f16)
    nc.scalar.activation(sqd[:], gTt[:], Act.Square,
                         accum_out=S["rhs_ge"][:, 0:1])

    # solve: cast accums to f16 (1e-3 scalar rel err, acceptable), then a
    # f16 ones-MM (44 ns vs 444 fp32) broadcasts damping+||g||^2, -STEP*e
    nc.vector.tensor_scalar_mul(S["rhs16"][:], S["rhs_ge"][:], 1.0)
    nc.tensor.matmul(p_ge[:], S["ones"][:], S["rhs16"][:],
                     start=True, stop=True)
    recipT = t("recipT", [128, 1], f32)
    upd = t("upd", [128, _B], f32)
    nc.vector.reciprocal(recipT[:], p_ge[:, 0:1])
    nc.vector.tensor_scalar(upd[:], gT, recipT[:], p_ge[:, 1:2],
                            op0=Alu.mult, op1=Alu.mult)
    nc.vector.tensor_tensor(flatT[:], flatT[:], upd[:], op=Alu.add)


def _declare_io(nc, mybir, precs):
    f32 = mybir.dt.float32
    f16 = mybir.dt.float16
    bf16 = mybir.dt.bfloat16
    D = {}
    specs = [("flatT0", [_HA, _B], f32),
             ("DE", [1, 1], f32)]
    if precs[0] == "p":
        specs += [("ACT0H", [128, _B], bf16), ("ACT0L", [128, _B], bf16)]
    if ("q" in precs) or ("p" in precs):
        specs.append(("BLOBB", [128, _CB], bf16))
    if "h" in precs:
        specs.append(("BLOB16", [128, _C16], f16))
    for name, shape, dt in specs:
        D[name] = nc.dram_tensor(name, shape, dt, kind="ExternalInput").ap()
    OUT = nc.dram_tensor("flatT_out", [_HA, _B], f32,
                         kind="ExternalOutput").ap()
    return D, OUT


def _build(precs=_PRECS, iters=None):
    import concourse.bacc as bacc
    import concourse.mybir as mybir
    from concourse import tile

    precs = list(precs if iters is None else (precs * iters)[:iters])
    nc = bacc.Bacc("TRN2", target_bir_lowering=False, debug=False,
                   num_devices=_N_CORES)
    D, OUT = _declare_io(nc, mybir, precs)
    with tile.TileContext(nc) as tc:
        with (
            tc.tile_pool(name="sb", bufs=1) as sb,
            tc.tile_pool(name="ps", bufs=1, space="PSUM") as ps,
        ):
            S = _emit_state(nc, tc, sb, ps, D, mybir, precs)
            for i, prec in enumerate(precs):
                nxt = precs[i + 1] if i + 1 < len(precs) else prec
                _emit_iter(nc, S, sb, mybir, prec, nxt, first=(i == 0),
                           emit_next_bias1=(i + 1 < len(precs)))
            nc.sync.dma_start(OUT, S["flatT"][0:_HA, :])
    nc.compile()
    return nc


def _stackT(x_bf):
    """[B, 512] -> [128, 128] T-stacked: out[p, 32c+b] = x[b, 128c+p]."""
    out = np.empty((128, 128), dtype=np.float32)
    for c in range(4):
        out[:, 32*c:32*c+32] = x_bf[:, 128*c:128*(c+1)].T
    return out


def _chunk128(Wkm, X):
    """[512, X] -> [128, 4*X] k-major blocks."""
    return Wkm.reshape(4, 128, X).transpose(1, 0, 2).reshape(128, 4*X)


def _host_prep(init_actions, z, W1, b1, W2, b2, W3, b3, precs=_PRECS):
    f = np.float32
    h = np.float16
    init_actions = np.ascontiguousarray(init_actions, dtype=f)
    z = np.ascontiguousarray(z, dtype=f)
    W1 = np.ascontiguousarray(W1, dtype=f)
    b1 = np.ascontiguousarray(b1, dtype=f)
    W2 = np.ascontiguousarray(W2, dtype=f)
    b2 = np.ascontiguousarray(b2, dtype=f)
    W3 = np.ascontiguousarray(W3, dtype=f)
    b3 = np.ascontiguousarray(b3, dtype=f)

    W1z, W1a = W1[:_SZ], W1[_SZ:]
    c1 = (z @ W1z + b1).astype(f)                     # [B, 512] constant
    w3 = W3[:, 0]
    W2TWf = (W2.T * (-w3 / _B)[:, None]).astype(f)    # [512(f2), 512(f1)]
    W3Cf = np.ascontiguousarray(w3.reshape(4, 128).T)  # [128, 4]
    W1aTp = np.zeros((512, 128), dtype=f)
    W1aTp[:, :_HA] = W1a.T

    def bf(x):
        import ml_dtypes
        return x.astype(ml_dtypes.bfloat16)

    w3r = W3Cf.astype(h).astype(f)
    E0P = _STEP * (b3[0] - w3r.sum(dtype=f) - _OFF)

    c1bT = _stackT(c1)
    eye = np.eye(128, dtype=f)

    def hilo(x, cast):
        xh = cast(x)
        xl = cast(x - xh.astype(f))
        return xh, xl

    ins = {
        "flatT0": np.ascontiguousarray(init_actions.T),
        "DE": np.array([[E0P / _B]], dtype=f),
    }
    if precs[0] == "p":
        import ml_dtypes
        f0 = np.zeros((128, _B), dtype=f)
        f0[0:_HA, :] = init_actions.T
        a0h = f0.astype(ml_dtypes.bfloat16)
        a0l = (f0 - a0h.astype(f)).astype(ml_dtypes.bfloat16)
        ins["ACT0H"] = a0h
        ins["ACT0L"] = a0l

    if ("q" in precs) or ("p" in precs):
        W2hi = bf(W2)
        W2lo = bf(W2 - W2hi.astype(f))
        W1ahi = bf(W1a)
        W1alo = bf(W1a - W1ahi.astype(f))
        W2TWhi = bf(W2TWf)
        W2TWlo = bf(W2TWf - W2TWhi.astype(f))
        W1aTphi = bf(W1aTp)
        W1aTplo = bf(W1aTp - W1aTphi.astype(f))
        b2bT_q = _stackT(np.broadcast_to(
            (b2 - (W2hi.astype(f) + W2lo.astype(f)).sum(axis=0, dtype=f)
             ).astype(f), (_B, _NF)))
        blobb = np.zeros((128, _CB), dtype=W2hi.dtype)
        blobb[0:_HA, _CB_W1AH:_CB_W1AH + _NF] = W1ahi
        blobb[0:_HA, _CB_W1AL:_CB_W1AL + _NF] = W1alo
        blobb[:, _CB_W2H:_CB_W2H + 4*_NF] = _chunk128(W2hi, _NF)
        blobb[:, _CB_W2L:_CB_W2L + 4*_NF] = _chunk128(W2lo, _NF)
        blobb[:, _CB_W2TWH:_CB_W2TWH + 4*_NF] = _chunk128(W2TWhi, _NF)
        blobb[:, _CB_W2TWL:_CB_W2TWL + 4*_NF] = _chunk128(W2TWlo, _NF)
        blobb[:, _CB_W1ATPH:_CB_W1ATPH + 512] = _chunk128(W1aTphi, 128)
        blobb[:, _CB_W1ATPL:_CB_W1ATPL + 512] = _chunk128(W1aTplo, 128)
        blobb[:, _CB_W3CH:_CB_W3CH + 4] = bf(W3Cf)
        blobb[:, _CB_EYE:_CB_EYE + 128] = bf(eye)
        c1h, c1l = hilo(c1bT, bf)
        b2h, b2l = hilo(b2bT_q, bf)
        blobb[:, _CB_C1H:_CB_C1H + 128] = c1h
        blobb[:, _CB_C1L:_CB_C1L + 128] = c1l
        blobb[:, _CB_B2H:_CB_B2H + 128] = b2h
        blobb[:, _CB_B2L:_CB_B2L + 128] = b2l
        ins["BLOBB"] = blobb

    if "h" in precs:
        W1ah = W1a.astype(h)
        blob16 = np.zeros((128, _C16), dtype=h)
        blob16[0:_HA, _C16_W1A:_C16_W1A + _NF] = W1ah
        blob16[:, _C16_W2:_C16_W2 + 4*_NF] = _chunk128(W2, _NF).astype(h)
        blob16[:, _C16_W2TW:_C16_W2TW + 4*_NF] = _chunk128(
            W2TWf, _NF).astype(h)
        blob16[:, _C16_W1ATP:_C16_W1ATP + 512] = _chunk128(
            W1aTp, 128).astype(h)
        blob16[:, _C16_W3C:_C16_W3C + 4] = W3Cf.astype(h)
        blob16[:, _C16_EYE:_C16_EYE + 128] = eye.astype(h)
        b2bT_h16 = _stackT(np.broadcast_to(
            (b2 - W2.astype(h).astype(f).sum(axis=0, dtype=f)).astype(f),
            (_B, _NF)))
        c1h16, c1l16 = hilo(c1bT, lambda x: x.astype(h))
        b2h16, b2l16 = hilo(b2bT_h16, lambda x: x.astype(h))
        blob16[:, _C16_C1H:_C16_C1H + 128] = c1h16
        blob16[:, _C16_C1L:_C16_C1L + 128] = c1l16
        blob16[:, _C16_B2H:_C16_B2H + 128] = b2h16
        blob16[:, _C16_B2L:_C16_B2L + 128] = b2l16
        ins["BLOB16"] = blob16

    return ins


def kernel(init_actions, z, W1, b1, W2, b2, W3, b3):
    from concourse import bass_utils

    key = ("nc", _PRECS)
    if key not in _CACHE:
        _CACHE[key] = _build(precs=_PRECS)
    nc = _CACHE[key]

    ins = _host_prep(init_actions, z, W1, b1, W2, b2, W3, b3, precs=_PRECS)
    in_maps = [dict(ins) for _ in range(_N_CORES)]
    res = bass_utils.run_bass_kernel_spmd(nc, in_maps,
                                          core_ids=list(range(_N_CORES)))
    flatT = res.results[0]["flatT_out"]            # [112, 32]
    out = flatT.T.reshape(_B, _HH, _AA)
    return np.ascontiguousarray(out, dtype=np.float32)


# revision 6
# speedup vs baseline: 1.0588x; 1.0588x over previous
"""TRN2 Bass kernel v4 for nn_CVAEWithTrajectoryOptimization.

Same math as the v3 baseline (Sherman-Morrison LM: delta =
-e*g/(damping+||g||^2), 8 serial fwd+bwd MLP iterations), rebuilt around
HW-measured instruction costs (For_i trip-count-slope microbenches):
  - dependent (chained) DVE/ACT ops cost ~230-430 ns EACH regardless of
    engine; cross-engine hops add nothing beyond that -> the only levers
    are CHAIN OP COUNT and full-128-partition shapes (sub-128 partition
    DVE ops pay ~200 ns extra; everything action-space is padded 112->128).
  - MM instruction cost: f16/bf16 with 128-col stationary (FWL)
    ~44/34 ns, 112-col stationary ~141 ns, fp32 ~444 ns -> no fp32 MMs
    anywhere, all stationaries padded to 128 cols.
Design:
  - biases enter through the PE: identity-stationary matmuls stream the
    f16/bf16 hi+lo split of c1bT / b2bT into the same PSUM accumulation
    group as the weight matmuls, so t1/t2 are assembled entirely on PE
    and ACT reads PSUM directly (no DVE bias-add chain ops).
  - elu' gate em = exp(min(t,0)) = Exp(-Relu(-t)): two in-order ACT ops
    reading PSUM; h = elu(t)+1 = max(t+1, em) (one DVE op; t+1 prepared
    off-chain; the +1 is absorbed into b2bT = b2 - colsum(W2)).
  - ||g||^2 + damping in ONE chained ACT op: Square with accum_out over
    gT[128,33] whose 33rd column holds sqrt(DAMP/128); e-path offset E0P
    rides the escr Identity-activation bias; ones-matmul broadcasts both
    column sums to all partitions (partition-reduce + bcast in one MM).
  - precision schedule 'phhhhhhh': iteration 0 (the bifurcation-critical
    one: |upd| up to 3.6 parks ~1/3 of actions just past the +-1 clip
    boundary) runs bf16 STATIONARY hi/lo x STREAM hi/lo triplet matmuls
    (~fp32 grade, rel 4.8e-3 vs 6.0e-3 for v3's fp32-x2 schedule) at
    ~1/10 the fp32 MM cost; iterations 1-7 are f16 (f16-solo = 1.9e-2,
    bf16-solo streams = 2.4e-2 -> measured, not guessed).
  - Exp/Relu/Abs/Sign/Identity/Square all live in the one
    'exp_and_others' ACT table set -> no table reloads.
  - clip-grad mask emitted after em1 (it head-of-line blocked a1 on the
    in-order ACT queue); iteration-0's stream hi/lo split is computed
    HOST-side (init_actions is an input) and DMA'd; the solve accums are
    cast once to f16 for a 44 ns f16 ones-MM (vs 444 ns fp32).
  - PSUM constraint (hard-won): accumulation groups must stay contiguous
    per bank region -- pre-opening several regions' bias groups to hoist
    them into stall windows silently corrupts results.
Measured (For_i trip-count slope, marginal ns per full kernel exec,
8-core SPMD): v3 baseline 132.9 us (early session) / 88-118 us (later
machine states); this kernel 70.8-83 us; paired same-session ratios
0.56-0.82, median ~0.76.
Replicated on all 8 cores (serial latency-bound chain; collectives would
dominate any sharding win).
"""
import os
import numpy as np

_B, _HH, _AA = 32, 16, 7
_HA = _HH * _AA          # 112
_SZ = 576
_NF = 512
_DAMP, _STEP, _ITERS, _OFF = 0.1, 0.1, 8, 1000.0
_N_CORES = 8
_PRIO_LOW = 1_500_000_000

# precision schedule: 'p' = bf16 hi/lo stationary pairs + hi/lo split
# streams (~fp32), 'q' = bf16 pairs + single-bf16 streams, 'h' = f16
_PRECS = os.environ.get("V4_PRECS", "phhhhhhh")

# f16 blob columns
_C16_W1A = 0
_C16_W2 = 512
_C16_W2TW = 2560
_C16_W1ATP = 4608
_C16_W3C = 5120
_C16_EYE = 5124
_C16_C1H = 5252
_C16_C1L = 5380
_C16_B2H = 5508
_C16_B2L = 5636
_C16 = 5764
# bf16 pair blob columns (hi interleaved before lo per tensor)
_CB_W1AH = 0
_CB_W1AL = 512
_CB_W2H = 1024
_CB_W2L = 3072
_CB_W2TWH = 5120
_CB_W2TWL = 7168
_CB_W1ATPH = 9216
_CB_W1ATPL = 9728
_CB_W3CH = 10240
_CB_EYE = 10244
_CB_C1H = 10372
_CB_C1L = 10500
_CB_B2H = 10628
_CB_B2L = 10756
_CB = 10884
# f32 blob columns (tiny)
_CF = 0

_CACHE = {}


def _dchunks(total, n):
    """Split [0,total) into n contiguous col ranges."""
    step = (total + n - 1) // n
    return [(i, min(i + step, total)) for i in range(0, total, step)]


def _emit_state(nc, tc, sb, ps, D, mybir, precs):
    f32 = mybir.dt.float32
    f16 = mybir.dt.float16
    bf16 = mybir.dt.bfloat16
    S = {}
    # flatT padded to 128 partitions (rows 112-127 stay zero): sub-128
    # partition-dim DVE ops cost ~434 ns vs ~231 ns at 128.
    S["flatT"] = sb.tile([128, _B], f32, tag="flatT", name="flatT")
    nc.vector.memset(S["flatT"][:], 0.0)
    nc.sync.dma_start(S["flatT"][0:_HA, :], D["flatT0"])
    if precs[0] == "p":
        # iteration 0's stream hi/lo split comes straight from the host
        # (init_actions is an input; |init| < 1 so clip is identity)
        S["act0h"] = sb.tile([128, _B], bf16, tag="act0h", name="act0h")
        S["act0l"] = sb.tile([128, _B], bf16, tag="act0l", name="act0l")
        nc.scalar.dma_start(S["act0h"][:], D["ACT0H"])
        nc.scalar.dma_start(S["act0l"][:], D["ACT0L"])

    queues = [nc.sync, nc.scalar, nc.gpsimd]
    qi = 0

    def q_dma(dst, src):
        nonlocal qi
        queues[qi % len(queues)].dma_start(dst, src)
        qi += 1

    has_q = ("q" in precs) or ("p" in precs)
    if has_q:
        blobb = sb.tile([128, _CB], bf16, tag="blobb", name="blobb")
        for a, b in _dchunks(_CB, 10):
            q_dma(blobb[:, a:b], D["BLOBB"][:, a:b])
        S["w1a_qh"] = blobb[:, _CB_W1AH:_CB_W1AH + _NF]
        S["w1a_ql"] = blobb[:, _CB_W1AL:_CB_W1AL + _NF]
        S["w2_qh"] = [blobb[:, _CB_W2H + _NF*k:_CB_W2H + _NF*(k+1)]
                      for k in range(4)]
        S["w2_ql"] = [blobb[:, _CB_W2L + _NF*k:_CB_W2L + _NF*(k+1)]
                      for k in range(4)]
        S["w2tw_qh"] = [blobb[:, _CB_W2TWH + _NF*k:_CB_W2TWH + _NF*(k+1)]
                        for k in range(4)]
        S["w2tw_ql"] = [blobb[:, _CB_W2TWL + _NF*k:_CB_W2TWL + _NF*(k+1)]
                        for k in range(4)]
        S["w1atp_qh"] = [blobb[:, _CB_W1ATPH + 128*k:_CB_W1ATPH + 128*(k+1)]
                         for k in range(4)]
        S["w1atp_ql"] = [blobb[:, _CB_W1ATPL + 128*k:_CB_W1ATPL + 128*(k+1)]
                         for k in range(4)]
        S["w3c_q"] = blobb[:, _CB_W3CH:_CB_W3CH + 4]
        S["eye_q"] = blobb[:, _CB_EYE:_CB_EYE + 128]
        S["c1h_q"] = blobb[:, _CB_C1H:_CB_C1H + 128]
        S["c1l_q"] = blobb[:, _CB_C1L:_CB_C1L + 128]
        S["b2h_q"] = blobb[:, _CB_B2H:_CB_B2H + 128]
        S["b2l_q"] = blobb[:, _CB_B2L:_CB_B2L + 128]

    if "h" in precs:
        blob16 = sb.tile([128, _C16], f16, tag="blob16", name="blob16")
        for a, b in _dchunks(_C16, 6):
            q_dma(blob16[:, a:b], D["BLOB16"][:, a:b])
        S["w1a_h"] = blob16[:, _C16_W1A:_C16_W1A + _NF]
        S["w2_h"] = [blob16[:, _C16_W2 + _NF*k:_C16_W2 + _NF*(k+1)]
                     for k in range(4)]
        S["w2tw_h"] = [blob16[:, _C16_W2TW + _NF*k:_C16_W2TW + _NF*(k+1)]
                       for k in range(4)]
        S["w1atp_h"] = [blob16[:, _C16_W1ATP + 128*k:_C16_W1ATP + 128*(k+1)]
                        for k in range(4)]
        S["w3c_h"] = blob16[:, _C16_W3C:_C16_W3C + 4]
        S["eye_h"] = blob16[:, _C16_EYE:_C16_EYE + 128]
        S["c1h_h"] = blob16[:, _C16_C1H:_C16_C1H + 128]
        S["c1l_h"] = blob16[:, _C16_C1L:_C16_C1L + 128]
        S["b2h_h"] = blob16[:, _C16_B2H:_C16_B2H + 128]
        S["b2l_h"] = blob16[:, _C16_B2L:_C16_B2L + 128]

    S["ones"] = sb.tile([128, 128], f32, tag="ones", name="ones")
    nc.vector.memset(S["ones"][:], 1.0)
    # rhs_ge [128, 2] f32: col0 = per-partition sum(g^2) partials (ACT
    # Square accum over gT[128,33]; gT col 32 holds sqrt(DAMP/128) so the
    # damping rides the same accumulation); col1 row0 = sum(p_r)*STEP/B
    # + E0P (E0P rides the escr bias).  ones-MM broadcasts col sums.
    S["rhs_ge"] = sb.tile([128, 2], f32, tag="rhs_ge", name="rhs_ge")
    nc.vector.memset(S["rhs_ge"][:], 0.0)
    S["deb"] = sb.tile([1, 1], f32, tag="deb", name="deb")
    nc.sync.dma_start(S["deb"][:], D["DE"])
    S["gT"] = sb.tile([128, _B + 1], f32, tag="gT", name="gT")
    nc.vector.memset(S["gT"][:, _B:_B+1],
                     float(np.sqrt(_DAMP / 128.0)))

    S["p_h1"] = ps.tile([128, 128], f32, tag="p_h1", name="p_h1")
    S["p_h2"] = ps.tile([128, 128], f32, tag="p_h2", name="p_h2")
    S["p_g1"] = ps.tile([128, 128], f32, tag="p_g1", name="p_g1")
    S["p_ga"] = ps.tile([128, _B], f32, tag="p_ga", name="p_ga")
    S["p_r"] = ps.tile([1, _B], f32, tag="p_r", name="p_r")
    S["p_ge"] = ps.tile([128, 2], f32, tag="p_ge", name="p_ge")
    S["p_scr"] = ps.tile([_B, 1], f32, tag="p_scr", name="p_scr")
    S["nprio"] = 0

    # PE clock warm across the DMA window; ACT Exp table pre-warm
    warm_deps = [S["flatT"][0:128, 0:_B]]
    if has_q:
        warm_deps += [S["w1a_qh"][0:112, 0:32], S["w2_qh"][3][0:112, 0:32]]
    if "h" in precs:
        warm_deps += [S["w2_h"][3][0:112, 0:32]]
    for dep in warm_deps:
        for _ in range(8):
            _dummy_mm(nc, S, dep)
    warm = sb.tile([1, 1], f32, tag="actwarm", name="actwarm")
    a1 = nc.scalar.activation(warm[:], S["rhs_ge"][0:1, 0:1],
                              mybir.ActivationFunctionType.Exp)
    a1.bass_priority = _PRIO_LOW - 2
    return S


def _emit_bias1(nc, S, wp):
    """fwd1 bias: 2 identity-stationary MMs per m-region starting the
    p_h1 PSUM groups.  Emitted in the PREVIOUS iteration's tail window
    (PE is idle there) so fwd1 only runs 4 weight MMs after the clip."""
    eye = S[f"eye_{wp}"]
    for m in range(4):
        reg = S["p_h1"][:, 32*m:32*m+32]
        nc.tensor.matmul(reg, eye, S[f"c1h_{wp}"][:, 32*m:32*m+32],
                         start=True, stop=False)
        nc.tensor.matmul(reg, eye, S[f"c1l_{wp}"][:, 32*m:32*m+32],
                         start=False, stop=False)


def _dummy_mm(nc, S, dep):
    m = dep.shape[1] if len(dep.shape) > 1 else 1
    mm = nc.tensor.matmul(S["p_scr"][0:m, :], dep, dep[:, 0:1],
                          start=True, stop=True)
    mm.bass_priority = _PRIO_LOW + S["nprio"]
    S["nprio"] += 1
    return mm


def _emit_iter(nc, S, sb, mybir, prec, nxt_prec, first=False,
               emit_next_bias1=True):
    """One LM iteration.  prec: 'p' = bf16 pairs + split streams,
    'q' = bf16 pairs + single-bf16 streams, 'h' = f16.
    first=True: |init_actions| < 1 so the clip-grad mask is all-ones ->
    no mask, gT = p_ga directly."""
    f32 = mybir.dt.float32
    f16 = mybir.dt.float16
    bf16 = mybir.dt.bfloat16
    pair = prec in ("q", "p")
    split = prec == "p"
    wp = "q" if pair else "h"          # weight-blob key
    dt = bf16 if pair else f16
    Alu = mybir.AluOpType
    Act = mybir.ActivationFunctionType
    flatT = S["flatT"]
    p_h1, p_h2, p_g1 = S["p_h1"], S["p_h2"], S["p_g1"]
    p_ga, p_r, p_ge = S["p_ga"], S["p_r"], S["p_ge"]

    def t(name, shape, d):
        return sb.tile(shape, d, tag=f"{name}_{prec}", name=f"{name}_{prec}")

    def mm_pairs(psum_ap, stat_hi, stat_lo, stream, start, stop,
                 stream_lo=None):
        if not pair:
            nc.tensor.matmul(psum_ap, stat_hi, stream, start=start, stop=stop)
            return
        nc.tensor.matmul(psum_ap, stat_hi, stream, start=start, stop=False)
        if stream_lo is not None:
            nc.tensor.matmul(psum_ap, stat_hi, stream_lo, start=False,
                             stop=False)
        nc.tensor.matmul(psum_ap, stat_lo, stream, start=False, stop=stop)

    def w(name, k=None):
        if pair:
            if k is None:
                return S[f"{name}_qh"], S[f"{name}_ql"]
            return S[f"{name}_qh"][k], S[f"{name}_ql"][k]
        if k is None:
            return S[f"{name}_h"], None
        return S[f"{name}_h"][k], None

    def split_hl(x32, nm):
        """f32 -> (bf16 hi, bf16 lo) pair; lo=None when not splitting."""
        if not split:
            return None, None
        xh = t(nm + "H", list(x32.shape), bf16)
        xl = t(nm + "L", list(x32.shape), bf16)
        nc.vector.tensor_scalar_mul(xh[:], x32, 1.0)
        nc.vector.tensor_tensor(xl[:], x32, xh[:], op=Alu.subtract)
        return xh, xl

    # head: acts = clip(flat) -> stream dtype (hi/lo split for 'p')
    if split and first:
        actsT = S["act0h"]          # host-computed bf16 hi/lo of init
        actsL = S["act0l"]
    elif split:
        acts32 = t("acts32", [128, _B], f32)
        nc.vector.tensor_scalar(acts32[:], flatT[:], -1.0, 1.0,
                                op0=Alu.max, op1=Alu.min)
        actsT = t("actsT", [128, _B], dt)
        actsL = t("actsL", [128, _B], dt)
        nc.vector.tensor_scalar_mul(actsT[:], acts32[:], 1.0)
        nc.vector.tensor_tensor(actsL[:], acts32[:], actsT[:],
                                op=Alu.subtract)
    else:
        actsT = t("actsT", [128, _B], dt)
        actsL = None
        nc.vector.tensor_scalar(actsT[:], flatT[:], -1.0, 1.0,
                                op0=Alu.max, op1=Alu.min)

    # fwd1: per m-region, bias pair then weight MMs (PSUM allows only one
    # open accumulation group per bank region sequence -- keep contiguous)
    eye = S[f"eye_{wp}"]
    w1h, w1l = w("w1a")
    for m in range(4):
        reg = p_h1[:, 32*m:32*m+32]
        nc.tensor.matmul(reg, eye, S[f"c1h_{wp}"][:, 32*m:32*m+32],
                         start=True, stop=False)
        nc.tensor.matmul(reg, eye, S[f"c1l_{wp}"][:, 32*m:32*m+32],
                         start=False, stop=False)
        mm_pairs(reg,
                 w1h[:, 128*m:128*(m+1)],
                 w1l[:, 128*m:128*(m+1)] if w1l is not None else None,
                 actsT[:], start=False, stop=True,
                 stream_lo=actsL[:] if actsL is not None else None)

    # em1 = exp(min(t1,0)) = Exp(-Relu(-t1)); h1s = max(t1+1, em1)
    hdt = f32 if split else dt
    a1 = t("a1", [128, 128], f32)
    em1 = t("em1", [128, 128], hdt)
    t1p1 = t("t1p1", [128, 128], hdt)
    h1s = t("h1s", [128, 128], hdt)
    nc.scalar.activation(a1[:], p_h1[:], Act.Relu, scale=-1.0)
    nc.scalar.activation(em1[:], a1[:], Act.Exp, scale=-1.0)
    nc.vector.tensor_scalar_add(t1p1[:], p_h1[:], 1.0)
    nc.vector.tensor_tensor(h1s[:], t1p1[:], em1[:], op=Alu.max)
    h1sH, h1sL = split_hl(h1s[:], "h1s")
    if split:
        h1s = h1sH

    # clip-grad mask from the CURRENT flatT (emitted after em1 so it no
    # longer head-of-line blocks a1 on the in-order ACT queue; correctness
    # is positional: it reads flatT before this iteration's update)
    if not first:
        absT = t("absT", [128, _B], f32)
        sgnT = t("sgnT", [128, _B], f32)
        maskT = t("maskT", [128, _B], f16)
        nc.scalar.activation(absT[:], flatT[:], Act.Abs)
        nc.scalar.activation(sgnT[:], absT[:], Act.Sign, bias=1.0, scale=-1.0)
        nc.scalar.activation(maskT[:], sgnT[:], Act.Relu)

    # fwd2: per m-region, bias pair then weight MMs
    for m in range(4):
        reg = p_h2[:, 32*m:32*m+32]
        nc.tensor.matmul(reg, eye, S[f"b2h_{wp}"][:, 32*m:32*m+32],
                         start=True, stop=False)
        nc.tensor.matmul(reg, eye, S[f"b2l_{wp}"][:, 32*m:32*m+32],
                         start=False, stop=False)
        for k in range(4):
            h, lo = w("w2", k)
            mm_pairs(reg,
                     h[:, 128*m:128*(m+1)],
                     lo[:, 128*m:128*(m+1)] if lo is not None else None,
                     h1s[:, 32*k:32*k+32], start=False, stop=(k == 3),
                     stream_lo=h1sL[:, 32*k:32*k+32] if h1sL is not None
                     else None)

    # em2 = exp(min(t2,0))
    a2 = t("a2", [128, 128], f32)
    em2 = t("em2", [128, 128], hdt)
    nc.scalar.activation(a2[:], p_h2[:], Act.Relu, scale=-1.0)
    nc.scalar.activation(em2[:], a2[:], Act.Exp, scale=-1.0)
    em2H, em2L = split_hl(em2[:], "em2")
    em2s = em2H if split else em2

    # bwd2: p_g1 = W2TW^T-chunks @ em2   (W3/B scale folded host-side)
    for m in range(4):
        for k in range(4):
            h, lo = w("w2tw", k)
            mm_pairs(p_g1[:, 32*m:32*m+32],
                     h[:, 128*m:128*(m+1)],
                     lo[:, 128*m:128*(m+1)] if lo is not None else None,
                     em2s[:, 32*k:32*k+32], start=(k == 0), stop=(k == 3),
                     stream_lo=em2L[:, 32*k:32*k+32] if em2L is not None
                     else None)

    # reward path (off-chain): t2p1/h2s fill the DVE while PE runs bwd2
    t2p1 = t("t2p1", [128, 128], dt)
    h2s = t("h2s", [128, 128], dt)
    nc.vector.tensor_scalar_add(t2p1[:], p_h2[:], 1.0)
    nc.vector.tensor_tensor(h2s[:], t2p1[:], em2[:], op=Alu.max)
    w3 = S[f"w3c_{wp}"]
    for k in range(4):
        nc.tensor.matmul(p_r[:], w3[:, k:k+1], h2s[:, 32*k:32*k+32],
                         start=(k == 0), stop=(k == 3))
    # e-path on ACT: rhs_ge[0,1] = sum(p_r)*STEP/B + E0P (via bias)
    escr = t("escr", [1, _B], f32)
    nc.scalar.activation(escr[:], p_r[:], Act.Identity,
                         bias=S["deb"][:],
                         scale=float(np.float32(_STEP / _B)),
                         accum_out=S["rhs_ge"][0:1, 1:2])

    # bwd chain: gh1p = p_g1 * em1; bwd1; gT = p_ga * mask
    gh1p = t("gh1p", [128, 128], hdt)
    nc.vector.tensor_tensor(gh1p[:], p_g1[:], em1[:], op=Alu.mult)
    gh1pH, gh1pL = split_hl(gh1p[:], "gh1p")
    gh1ps = gh1pH if split else gh1p
    for k in range(4):
        h, lo = w("w1atp", k)
        mm_pairs(p_ga[:], h, lo, gh1ps[:, 32*k:32*k+32],
                 start=(k == 0), stop=(k == 3),
                 stream_lo=gh1pL[:, 32*k:32*k+32] if gh1pL is not None
                 else None)

    gTt = S["gT"]
    if first:
        nc.vector.tensor_scalar_mul(gTt[:, 0:_B], p_ga[:], 1.0)
    else:
        nc.vector.tensor_tensor(gTt[:, 0:_B], p_ga[:], maskT[:],
                                op=Alu.mult)
    gT = gTt[:, 0:_B]

    # ||g||^2 + DAMP via ACT Square accum over [128, 33] -> rhs_ge col0
    sqd = t("sqd", [128, _B + 1], f16)
    nc.scalar.activation(sqd[:], gTt[:], Act.Square,
                         accum_out=S["rhs_ge"][:, 0:1])

    # solve: f32 ones-MM broadcasts damping+||g||^2 and -STEP*e (444 ns
    # MM, but no cast op / extra hops; measured parity with the f16+cast
    # variant and keeps the solve scalars exact)
    nc.tensor.matmul(p_ge[:], S["ones"][:], S["rhs_ge"][:],
                     start=True, stop=True)
    recipT = t("recipT", [128, 1], f32)
    upd = t("upd", [128, _B], f32)
    nc.vector.reciprocal(recipT[:], p_ge[:, 0:1])
    nc.vector.tensor_scalar(upd[:], gT, recipT[:], p_ge[:, 1:2],
                            op0=Alu.mult, op1=Alu.mult)
    nc.vector.tensor_tensor(flatT[:], flatT[:], upd[:], op=Alu.add)


def _declare_io(nc, mybir, precs):
    f32 = mybir.dt.float32
    f16 = mybir.dt.float16
    bf16 = mybir.dt.bfloat16
    D = {}
    specs = [("flatT0", [_HA, _B], f32),
             ("DE", [1, 1], f32)]
    if precs[0] == "p":
        specs += [("ACT0H", [128, _B], bf16), ("ACT0L", [128, _B], bf16)]
    if ("q" in precs) or ("p" in precs):
        specs.append(("BLOBB", [128, _CB], bf16))
    if "h" in precs:
        specs.append(("BLOB16", [128, _C16], f16))
    for name, shape, dt in specs:
        D[name] = nc.dram_tensor(name, shape, dt, kind="ExternalInput").ap()
    OUT = nc.dram_tensor("flatT_out", [_HA, _B], f32,
                         kind="ExternalOutput").ap()
    return D, OUT


def _build(precs=_PRECS, iters=None):
    import concourse.bacc as bacc
    import concourse.mybir as mybir
    from concourse import tile

    precs = list(precs if iters is None else (precs * iters)[:iters])
    nc = bacc.Bacc("TRN2", target_bir_lowering=False, debug=False,
                   num_devices=_N_CORES)
    D, OUT = _declare_io(nc, mybir, precs)
    with tile.TileContext(nc) as tc:
        with (
            tc.tile_pool(name="sb", bufs=1) as sb,
            tc.tile_pool(name="ps", bufs=1, space="PSUM") as ps,
        ):
            S = _emit_state(nc, tc, sb, ps, D, mybir, precs)
            for i, prec in enumerate(precs):
                nxt = precs[i + 1] if i + 1 < len(precs) else prec
                _emit_iter(nc, S, sb, mybir, prec, nxt, first=(i == 0),
                           emit_next_bias1=(i + 1 < len(precs)))
            nc.sync.dma_start(OUT, S["flatT"][0:_HA, :])
    nc.compile()
    return nc


def _stackT(x_bf):
    """[B, 512] -> [128, 128] T-stacked: out[p, 32c+b] = x[b, 128c+p]."""
    out = np.empty((128, 128), dtype=np.float32)
    for c in range(4):
        out[:, 32*c:32*c+32] = x_bf[:, 128*c:128*(c+1)].T
    return out


def _chunk128(Wkm, X):
    """[512, X] -> [128, 4*X] k-major blocks."""
    return Wkm.reshape(4, 128, X).transpose(1, 0, 2).reshape(128, 4*X)


def _host_prep(init_actions, z, W1, b1, W2, b2, W3, b3, precs=_PRECS):
    f = np.float32
    h = np.float16
    init_actions = np.ascontiguousarray(init_actions, dtype=f)
    z = np.ascontiguousarray(z, dtype=f)
    W1 = np.ascontiguousarray(W1, dtype=f)
    b1 = np.ascontiguousarray(b1, dtype=f)
    W2 = np.ascontiguousarray(W2, dtype=f)
    b2 = np.ascontiguousarray(b2, dtype=f)
    W3 = np.ascontiguousarray(W3, dtype=f)
    b3 = np.ascontiguousarray(b3, dtype=f)

    W1z, W1a = W1[:_SZ], W1[_SZ:]
    c1 = (z @ W1z + b1).astype(f)                     # [B, 512] constant
    w3 = W3[:, 0]
    W2TWf = (W2.T * (-w3 / _B)[:, None]).astype(f)    # [512(f2), 512(f1)]
    W3Cf = np.ascontiguousarray(w3.reshape(4, 128).T)  # [128, 4]
    W1aTp = np.zeros((512, 128), dtype=f)
    W1aTp[:, :_HA] = W1a.T

    def bf(x):
        import ml_dtypes
        return x.astype(ml_dtypes.bfloat16)

    w3r = W3Cf.astype(h).astype(f)
    E0P = _STEP * (b3[0] - w3r.sum(dtype=f) - _OFF)

    c1bT = _stackT(c1)
    eye = np.eye(128, dtype=f)

    def hilo(x, cast):
        xh = cast(x)
        xl = cast(x - xh.astype(f))
        return xh, xl

    ins = {
        "flatT0": np.ascontiguousarray(init_actions.T),
        "DE": np.array([[E0P / _B]], dtype=f),
    }
    if precs[0] == "p":
        import ml_dtypes
        f0 = np.zeros((128, _B), dtype=f)
        f0[0:_HA, :] = init_actions.T
        a0h = f0.astype(ml_dtypes.bfloat16)
        a0l = (f0 - a0h.astype(f)).astype(ml_dtypes.bfloat16)
        ins["ACT0H"] = a0h
        ins["ACT0L"] = a0l

    if ("q" in precs) or ("p" in precs):
        W2hi = bf(W2)
        W2lo = bf(W2 - W2hi.astype(f))
        W1ahi = bf(W1a)
        W1alo = bf(W1a - W1ahi.astype(f))
        W2TWhi = bf(W2TWf)
        W2TWlo = bf(W2TWf - W2TWhi.astype(f))
        W1aTphi = bf(W1aTp)
        W1aTplo = bf(W1aTp - W1aTphi.astype(f))
        b2bT_q = _stackT(np.broadcast_to(
            (b2 - (W2hi.astype(f) + W2lo.astype(f)).sum(axis=0, dtype=f)
             ).astype(f), (_B, _NF)))
        blobb = np.zeros((128, _CB), dtype=W2hi.dtype)
        blobb[0:_HA, _CB_W1AH:_CB_W1AH + _NF] = W1ahi
        blobb[0:_HA, _CB_W1AL:_CB_W1AL + _NF] = W1alo
        blobb[:, _CB_W2H:_CB_W2H + 4*_NF] = _chunk128(W2hi, _NF)
        blobb[:, _CB_W2L:_CB_W2L + 4*_NF] = _chunk128(W2lo, _NF)
        blobb[:, _CB_W2TWH:_CB_W2TWH + 4*_NF] = _chunk128(W2TWhi, _NF)
        blobb[:, _CB_W2TWL:_CB_W2TWL + 4*_NF] = _chunk128(W2TWlo, _NF)
        blobb[:, _CB_W1ATPH:_CB_W1ATPH + 512] = _chunk128(W1aTphi, 128)
        blobb[:, _CB_W1ATPL:_CB_W1ATPL + 512] = _chunk128(W1aTplo, 128)
        blobb[:, _CB_W3CH:_CB_W3CH + 4] = bf(W3Cf)
        blobb[:, _CB_EYE:_CB_EYE + 128] = bf(eye)
        c1h, c1l = hilo(c1bT, bf)
        b2h, b2l = hilo(b2bT_q, bf)
        blobb[:, _CB_C1H:_CB_C1H + 128] = c1h
        blobb[:, _CB_C1L:_CB_C1L + 128] = c1l
        blobb[:, _CB_B2H:_CB_B2H + 128] = b2h
        blobb[:, _CB_B2L:_CB_B2L + 128] = b2l
        ins["BLOBB"] = blobb

    if "h" in precs:
        W1ah = W1a.astype(h)
        blob16 = np.zeros((128, _C16), dtype=h)
        blob16[0:_HA, _C16_W1A:_C16_W1A + _NF] = W1ah
        blob16[:, _C16_W2:_C16_W2 + 4*_NF] = _chunk128(W2, _NF).astype(h)
        blob16[:, _C16_W2TW:_C16_W2TW + 4*_NF] = _chunk128(
            W2TWf, _NF).astype(h)
        blob16[:, _C16_W1ATP:_C16_W1ATP + 512] = _chunk128(
            W1aTp, 128).astype(h)
        blob16[:, _C16_W3C:_C16_W3C + 4] = W3Cf.astype(h)
        blob16[:, _C16_EYE:_C16_EYE + 128] = eye.astype(h)
        b2bT_h16 = _stackT(np.broadcast_to(
            (b2 - W2.astype(h).astype(f).sum(axis=0, dtype=f)).astype(f),
            (_B, _NF)))
        c1h16, c1l16 = hilo(c1bT, lambda x: x.astype(h))
        b2h16, b2l16 = hilo(b2bT_h16, lambda x: x.astype(h))
        blob16[:, _C16_C1H:_C16_C1H + 128] = c1h16
        blob16[:, _C16_C1L:_C16_C1L + 128] = c1l16
        blob16[:, _C16_B2H:_C16_B2H + 128] = b2h16
        blob16[:, _C16_B2L:_C16_B2L + 128] = b2l16
        ins["BLOB16"] = blob16

    return ins


def kernel(init_actions, z, W1, b1, W2, b2, W3, b3):
    from concourse import bass_utils

    key = ("nc", _PRECS)
    if key not in _CACHE:
        _CACHE[key] = _build(precs=_PRECS)
    nc = _CACHE[key]

    ins = _host_prep(init_actions, z, W1, b1, W2, b2, W3, b3, precs=_PRECS)
    in_maps = [dict(ins) for _ in range(_N_CORES)]
    res = bass_utils.run_bass_kernel_spmd(nc, in_maps,
                                          core_ids=list(range(_N_CORES)))
    flatT = res.results[0]["flatT_out"]            # [112, 32]
    out = flatT.T.reshape(_B, _HH, _AA)
    return np.ascontiguousarray(out, dtype=np.float32)
